# revision 42
# baseline (speedup 1.0000x reference)
"""BSQ quantizer kernel for Trainium2, data-parallel across 8 NeuronCores.

Math notes (vs the JAX reference):
  - zq = sign(z) * Q (the straight-through estimator is a numerical no-op in
    the forward pass; sign(z/||z||) == sign(z)).
  - The per-group softmax over 512 sign codes factorizes into per-bit
    Bernoullis: prob_d = prod_c sigmoid(4*Q*s_dc*zn_c).  So
    avg_prob[g] = mean_n softmaxA_n (x) softmaxB_n with A = first 4 bits
    (16 values), B = last 5 bits (32 values); each side is a product of
    per-bit probabilities (automatically normalized), and the sum over
    samples of the outer product is a matmul contracted over samples.
  - per-bit entropy: -(p log p + (1-p) log(1-p)) with p = sigmoid(+-x)
    equals softplus(x) - x*sigmoid(x), x = 4*Q*zn.
  - commit loss per sample: sum_d (Q*sign - zn_d)^2 = 2 - 2*Q*sum_d |zn_d|.

Per-core layout: 16384 samples as [p=128 partitions, t=128 chunks, d=18],
sample n = p*128 + t, so sharding is a pure reshape on the host.
"""

import math

import numpy as np

import bass_rust
import concourse.bass as bass
import concourse.tile as tile
from concourse import mybir
from concourse.bass_utils import run_bass_kernel_spmd

AF = mybir.ActivationFunctionType
ALU = mybir.AluOpType
F32 = mybir.dt.float32
BF16 = mybir.dt.bfloat16
FP16 = mybir.dt.float16
I32 = mybir.dt.int32

N_CORES = 8
B, L, D = 32, 4096, 18
N = B * L                    # 131072 samples
NC_SAMP = N // N_CORES       # 16384 per core
P = 128                      # partitions
T = NC_SAMP // P             # 128 t-chunks per core
Q = 1.0 / math.sqrt(D)
# (start, size) chunks of t for the final tree stages + matmuls; the last
# chunk is small to shorten the PE tail after the last DVE op
CHUNKS = [(0, 20), (20, 36), (56, 36), (92, 28), (120, 8)]
IDX_OFF = (2.0**D - 1.0) / 2.0   # 131071.5
USE_SILU = True                  # HW has Silu; CoreSim doesn't


def _split_waits(nc, maxw=1):
    """walrus in this container only lowers one sync-wait per instruction;
    move excess waits onto freshly inserted NOPs just before the offender."""
    nsplit = 0
    for bb in nc.main_func.blocks:
        lst = bb.instructions
        i = 0
        while i < len(lst):
            ins = lst[i]
            si = ins.sync_info
            if si is not None and len(si.on_wait) > maxw:
                waits = list(si.on_wait)
                extra = waits[:-maxw]
                si.on_wait = waits[-maxw:]
                nops = []
                for j in range(0, len(extra), maxw):
                    nsplit += 1
                    nop = mybir.InstNoOp(name=f"I-waitsplit-{nsplit}")
                    nop.engine = ins.engine
                    nop.sync_info = bass_rust.SyncInfo(
                        on_wait=extra[j : j + maxw], on_update=[]
                    )
                    nops.append(nop)
                for k, nop in enumerate(nops):
                    lst.insert(i + k, nop)
                i += len(nops)
            i += 1
    return nsplit


def _ap(t, extra_offset, dims):
    """AP over tile t's tensor: partition dim copied from t, free dims given
    explicitly as [step, count] (steps in elements)."""
    return bass.AP(tensor=t.tensor, offset=t.offset + extra_offset, ap=[t.ap[0]] + dims)


def _build_nc(split_waits=True):
    nc = bass.Bass()
    z_in = nc.dram_tensor("z", [P, T * D], F32, kind="ExternalInput")
    basis_in = nc.dram_tensor("basis", [D], F32, kind="ExternalInput")
    zq_out = nc.dram_tensor("zq", [P, T * D], F32, kind="ExternalOutput")
    idx_out = nc.dram_tensor("idx", [P, T], I32, kind="ExternalOutput")
    sums_out = nc.dram_tensor("sums", [P, 3], F32, kind="ExternalOutput")
    avgp_out = nc.dram_tensor("avgp", [64, 32], F32, kind="ExternalOutput")

    with tile.TileContext(nc) as tc:
        with (
            tc.tile_pool(name="main", bufs=1) as pool,
            tc.tile_pool(name="psum", bufs=1, space="PSUM") as psum,
        ):
            zt = pool.tile([P, T, D], F32)          # [p, t, d]
            nc.sync.dma_start(out=zt, in_=z_in[:, :].rearrange("p (t d) -> p t d", d=D))
            basis2 = pool.tile([P, D], F32)          # 2^(15-d), replicated rows
            nc.sync.dma_start(
                out=basis2,
                in_=bass.AP(tensor=basis_in, offset=0, ap=[[0, P], [1, D]]),
            )

            s1 = pool.tile([P, T, D], F32)          # scratch
            sums = pool.tile([P, 3], F32)

            # ---- normalization chain, split into two t-halves so the
            # first half's sigmoid (and therefore the tree + PE stream)
            # starts while the second half is still normalizing ----
            H = T // 2
            ssq = pool.tile([P, T], F32)
            ln_s = pool.tile([P, T], F32)
            rn = pool.tile([P, T], F32)
            zn = pool.tile([P, T, D], F32)
            pq = pool.tile([P, 2, D, T], F32)       # [p, sign(0:-,1:+), d, t]
            halves = [(h * H, H) for h in range(2)]
            for lo, n in halves:
                nc.scalar.activation(s1[:, lo:lo + n], zt[:, lo:lo + n], AF.Square)
            for lo, n in halves:
                nc.vector.tensor_reduce(
                    out=ssq[:, lo:lo + n], in_=s1[:, lo:lo + n],
                    axis=mybir.AxisListType.X, op=ALU.add,
                )
            rn_acts = []
            for lo, n in halves:
                nc.scalar.activation(ln_s[:, lo:lo + n], ssq[:, lo:lo + n], AF.Ln)
                rn_acts.append(nc.scalar.activation(
                    rn[:, lo:lo + n], ln_s[:, lo:lo + n], AF.Exp, scale=-0.5
                ))
            zn_halves = []
            for lo, n in halves:
                zn_halves.append(nc.vector.tensor_mul(
                    zn[:, lo:lo + n], zt[:, lo:lo + n],
                    _ap(rn, lo, [[1, n], [0, D]]),
                ))
            zn_i = zn_halves[-1]
            # ---- sigmoid set ----
            sigs = []
            for h, (lo, n) in enumerate(halves):
                sig = nc.scalar.activation(
                    _ap(pq, D * T + lo, [[1, n], [T, D]]),
                    zn[:, lo:lo + n], AF.Sigmoid, scale=4.0 * Q,
                )
                sigs.append(sig)
                # p_minus = 1 - p_plus on DVE (2x)
                nc.vector.tensor_scalar(
                    out=_ap(pq, lo, [[T, D], [1, n]]),
                    in0=_ap(pq, D * T + lo, [[T, D], [1, n]]),
                    scalar1=-1.0, scalar2=1.0, op0=ALU.mult, op1=ALU.add,
                )
            # keep the natural_log block (h2's Ln/Exp) ahead of the sigmoids
            bass._add_dep_helper(
                sigs[0].ins, rn_acts[-1].ins, sync=False, reason="act set order"
            )
            sig_p = sigs[-1]

            # sign path on DVE: fills the window while ACT runs the sigmoid
            cmp = pool.tile([P, T, D], FP16)        # 1.0 if z>0 else 0.0
            nc.vector.tensor_scalar(
                out=cmp, in0=zt, scalar1=0.0, scalar2=None, op0=ALU.is_gt
            )
            zqt = pool.tile([P, T, D], F32)
            zq_i = nc.vector.tensor_scalar(
                out=zqt, in0=cmp, scalar1=2.0 * Q, scalar2=-Q,
                op0=ALU.mult, op1=ALU.add,
            )
            nc.sync.dma_start(
                out=zq_out[:, :].rearrange("p (t d) -> p t d", d=D), in_=zqt
            )

            # ---- Bernoulli product tree + avg_prob matmuls, chunked ----
            # pq element (s, d, t) at s*D*T + d*T + t; tree tiles are t-inner
            t16 = pool.tile([P, 32, T], FP16)       # [p, (g,i4), t]
            t32 = pool.tile([P, 64, T], FP16)       # [p, (g,j5), t]
            p01 = pool.tile([P, 2, 4, T], FP16)     # [p, g, (b0,b1), t]
            p23 = pool.tile([P, 2, 4, T], FP16)
            q45 = pool.tile([P, 2, 4, T], FP16)
            q67 = pool.tile([P, 2, 4, T], FP16)
            q678 = pool.tile([P, 2, 8, T], FP16)
            pss = [psum.tile([64, 32], F32, name=f"ps{c}") for c in range(len(CHUNKS))]

            def pair(dst, d_hi, d_lo, g, lo, n):
                """dst[g, b_hi, b_lo, t] = pq[b_hi, d_hi, t] * pq[b_lo, d_lo, t]"""
                nc.vector.tensor_mul(
                    _ap(dst, g * 4 * T + lo, [[2 * T, 2], [T, 2], [1, n]]),
                    _ap(pq, (9 * g + d_hi) * T + lo, [[D * T, 2], [0, 2], [1, n]]),
                    _ap(pq, (9 * g + d_lo) * T + lo, [[0, 2], [D * T, 2], [1, n]]),
                )

            # tree stages: a small t-prefix first so the PE matmul stream
            # starts early, then the bulk in one low-overhead pass
            C0 = CHUNKS[0][1]

            def smalls(lo, n):
                for g in range(2):
                    pair(p01, 0, 1, g, lo, n)
                    pair(p23, 2, 3, g, lo, n)
                    pair(q45, 4, 5, g, lo, n)
                    pair(q67, 6, 7, g, lo, n)
                    # q678[(i67, b8), t] = q67[i67, t] * pq[b8, d=9g+8, t]
                    nc.vector.tensor_mul(
                        _ap(q678, g * 8 * T + lo, [[2 * T, 4], [T, 2], [1, n]]),
                        _ap(q67, g * 4 * T + lo, [[T, 4], [0, 2], [1, n]]),
                        _ap(pq, (9 * g + 8) * T + lo, [[0, 4], [D * T, 2], [1, n]]),
                    )

            def finals(c0, cs):
                for g in range(2):
                    # t16[(g, i01, i23), t] = p01[i01, t] * p23[i23, t]
                    nc.vector.tensor_mul(
                        _ap(t16, g * 16 * T + c0, [[4 * T, 4], [T, 4], [1, cs]]),
                        _ap(p01, g * 4 * T + c0, [[T, 4], [0, 4], [1, cs]]),
                        _ap(p23, g * 4 * T + c0, [[0, 4], [T, 4], [1, cs]]),
                    )
                    # t32[(g, i45, i678), t] = q45[i45, t] * q678[i678, t]
                    last = nc.vector.tensor_mul(
                        _ap(t32, g * 32 * T + c0, [[8 * T, 4], [T, 8], [1, cs]]),
                        _ap(q45, g * 4 * T + c0, [[T, 4], [0, 8], [1, cs]]),
                        _ap(q678, g * 8 * T + c0, [[0, 4], [T, 8], [1, cs]]),
                    )
                return last

            def mms(c, c0, cs):
                for tt in range(c0, c0 + cs):
                    nc.tensor.matmul(
                        pss[c],
                        _ap(t32, tt, [[T, 64]]),   # [K=128, 64] stationary
                        _ap(t16, tt, [[T, 32]]),   # [K=128, 32] moving
                        start=(tt == c0),
                        stop=(tt == c0 + cs - 1),
                    )

            avg_sb = pool.tile([64, 32], F32)
            smalls(0, C0)
            finals(0, C0)
            mms(0, 0, C0)
            nc.scalar.copy(avg_sb, pss[0])
            smalls(CHUNKS[1][0], CHUNKS[1][1])
            finals(CHUNKS[1][0], CHUNKS[1][1])
            mms(1, CHUNKS[1][0], CHUNKS[1][1])
            nc.vector.tensor_add(avg_sb, avg_sb, pss[1])
            smalls(CHUNKS[2][0], T - CHUNKS[2][0])
            last_fin = None
            for c, (c0, cs) in enumerate(CHUNKS):
                if c < 2:
                    continue
                last_fin = finals(c0, cs)
                mms(c, c0, cs)
                nc.vector.tensor_add(avg_sb, avg_sb, pss[c])

            # ---- entropy + commit partial sums ----
            if USE_SILU:
                sl1 = pool.tile([P, D, T], F32)
                silu_i = nc.scalar.activation(      # sum of x*sigmoid(x), x=4Q*zn
                    sl1, zn, AF.Silu, scale=4.0 * Q, accum_out=sums[:, 1:2]
                )
                bass._add_dep_helper(
                    silu_i.ins, sig_p.ins, sync=False, reason="act table set order"
                )
            ex = pool.tile([P, D, T], F32)
            exp_e = nc.scalar.activation(ex, zn, AF.Exp, scale=4.0 * Q)   # e^x
            nc.scalar.activation(                                          # softplus
                s1, ex, AF.Ln, bias=1.0, accum_out=sums[:, 0:1]
            )
            ab2 = pool.tile([P, D, T], F32)
            nc.scalar.activation(                                          # 2Q|zn|
                ab2, zn, AF.Abs, scale=2.0 * Q, accum_out=sums[:, 2:3]
            )
            # keep Exp/Ln after the sigmoid/silu block (one table switch only)
            bass._add_dep_helper(
                exp_e.ins,
                (silu_i if USE_SILU else sig_p).ins,
                sync=False,
                reason="act table set order",
            )
            if not USE_SILU:
                # sum of zn*sigmoid(x) via DVE (sim has no Silu)
                sw1 = pool.tile([P, T, D], F32)
                nc.vector.tensor_mul(
                    sw1, zn, _ap(pq, D * T, [[1, T], [T, D]])
                )
                nc.vector.tensor_reduce(
                    out=sums[:, 1:2], in_=sw1, axis=mybir.AxisListType.XY, op=ALU.add
                )

            # idx = sum_d 2^(17-d)*cmp_d  (exact: powers of two)
            sw2 = pool.tile([P, T, D], FP16)
            idx_i = nc.vector.tensor_mul(sw2, cmp, _ap(basis2, 0, [[0, T], [1, D]]))
            bass._add_dep_helper(
                idx_i.ins, last_fin.ins, sync=False, reason="idx after tree"
            )
            bass._add_dep_helper(
                zq_i.ins, last_fin.ins, sync=False, reason="zq after tree"
            )
            idxf = pool.tile([P, T], F32)
            nc.vector.tensor_reduce(
                out=idxf, in_=sw2, axis=mybir.AxisListType.X, op=ALU.add
            )
            idxi = pool.tile([P, T], I32)
            nc.vector.tensor_scalar(
                out=idxi, in0=idxf, scalar1=4.0, scalar2=None, op0=ALU.mult
            )
            nc.sync.dma_start(out=idx_out[:, :], in_=idxi)

            nc.sync.dma_start(out=avgp_out[:, :], in_=avg_sb)
            nc.sync.dma_start(out=sums_out[:, :], in_=sums)

    if split_waits:
        _split_waits(nc)
    return nc


_NC_CACHE = None


def _get_nc():
    global _NC_CACHE
    if _NC_CACHE is None:
        _NC_CACHE = _build_nc()
    return _NC_CACHE


_BASIS2 = (2.0 ** np.arange(D - 3, -3, -1)).astype(np.float32)  # 2^(15-d), fp16-safe


def _shard(z):
    """z [B, L, D] f32 -> list of per-core input dicts (device layout)."""
    zf = np.ascontiguousarray(z, dtype=np.float32).reshape(N, D)
    ins = []
    for c in range(N_CORES):
        zc = zf[c * NC_SAMP : (c + 1) * NC_SAMP]          # [16384, 18]
        ins.append({"z": zc.reshape(P, T * D), "basis": _BASIS2})
    return ins


def _unshard(results):
    zq = np.empty((N, D), np.float32)
    idx = np.empty((N,), np.int32)
    sp_t = sw_t = cm_t = 0.0
    avg = np.zeros((64, 32), np.float64)
    for c, r in enumerate(results):
        zq[c * NC_SAMP : (c + 1) * NC_SAMP] = r["zq"].reshape(NC_SAMP, D)
        idx[c * NC_SAMP : (c + 1) * NC_SAMP] = r["idx"].reshape(NC_SAMP)
        s = r["sums"].astype(np.float64)
        sp_t += s[:, 0].sum()
        sw_t += s[:, 1].sum()
        cm_t += s[:, 2].sum()
        avg += r["avgp"].astype(np.float64)

    persample = (sp_t - (1.0 if USE_SILU else 4.0 * Q) * sw_t) / N
    commit = 2.0 - cm_t / N
    ap = np.empty((2, 512), np.float64)
    for g in range(2):
        # avg[(g,j), (g,i)] = sum_n b_j * a_i ; ap[g, i*32+j] = that / N
        ap[g] = (avg[g * 32 : (g + 1) * 32, g * 16 : (g + 1) * 16].T / N).reshape(512)
    cb_ent = float(-(ap * np.log(ap + 1e-8)).sum())
    total = commit + persample - cb_ent

    return (
        zq.reshape(B, L, D),
        np.float32(total),
        np.float32(persample),
        np.float32(cb_ent),
        np.float32(commit),
        idx.reshape(B, L),
    )


def kernel(z):
    nc = _get_nc()
    ins = _shard(np.asarray(z))
    res = run_bass_kernel_spmd(nc, ins, core_ids=list(range(N_CORES)))
    return _unshard(res.results)


# revision 44
# speedup vs baseline: 1.1311x; 1.1311x over previous
"""BSQ quantizer kernel for Trainium2, data-parallel across 8 NeuronCores.

Math notes (vs the JAX reference):
  - zq = sign(z) * Q (the straight-through estimator is a numerical no-op in
    the forward pass; sign(z/||z||) == sign(z)).
  - The per-group softmax over 512 sign codes factorizes into per-bit
    Bernoullis: prob_d = prod_c sigmoid(4*Q*s_dc*zn_c).  So
    avg_prob[g] = mean_n softmaxA_n (x) softmaxB_n with A = first 4 bits
    (16 values), B = last 5 bits (32 values); each side is a product of
    per-bit probabilities (automatically normalized), and the sum over
    samples of the outer product is a matmul contracted over samples.
  - per-bit entropy: -(p log p + (1-p) log(1-p)) with p = sigmoid(+-x)
    equals softplus(x) - x*sigmoid(x), x = 4*Q*zn.
  - commit loss per sample: sum_d (Q*sign - zn_d)^2 = 2 - 2*Q*sum_d |zn_d|.

Per-core layout: 16384 samples as [p=128 partitions, t=128 chunks, d=18],
sample n = p*128 + t, so sharding is a pure reshape on the host.
"""

import math

import numpy as np

import bass_rust
import concourse.bass as bass
import concourse.tile as tile
from concourse import mybir
from concourse.bass_utils import run_bass_kernel_spmd

AF = mybir.ActivationFunctionType
ALU = mybir.AluOpType
F32 = mybir.dt.float32
BF16 = mybir.dt.bfloat16
FP16 = mybir.dt.float16
I32 = mybir.dt.int32

N_CORES = 8
B, L, D = 32, 4096, 18
N = B * L                    # 131072 samples
NC_SAMP = N // N_CORES       # 16384 per core
P = 128                      # partitions
T = NC_SAMP // P             # 128 t-chunks per core
Q = 1.0 / math.sqrt(D)
# (start, size) chunks of t for the final tree stages + matmuls; the last
# chunk is small to shorten the PE tail after the last DVE op
CHUNKS = [(0, 16), (16, 52), (68, 52), (120, 8)]
IDX_OFF = (2.0**D - 1.0) / 2.0   # 131071.5
USE_SILU = True                  # HW has Silu; CoreSim doesn't


def _split_waits(nc, maxw=1):
    """walrus in this container only lowers one sync-wait per instruction;
    move excess waits onto freshly inserted NOPs just before the offender."""
    nsplit = 0
    for bb in nc.main_func.blocks:
        lst = bb.instructions
        i = 0
        while i < len(lst):
            ins = lst[i]
            si = ins.sync_info
            if si is not None and len(si.on_wait) > maxw:
                waits = list(si.on_wait)
                extra = waits[:-maxw]
                si.on_wait = waits[-maxw:]
                nops = []
                for j in range(0, len(extra), maxw):
                    nsplit += 1
                    nop = mybir.InstNoOp(name=f"I-waitsplit-{nsplit}")
                    nop.engine = ins.engine
                    nop.sync_info = bass_rust.SyncInfo(
                        on_wait=extra[j : j + maxw], on_update=[]
                    )
                    nops.append(nop)
                for k, nop in enumerate(nops):
                    lst.insert(i + k, nop)
                i += len(nops)
            i += 1
    return nsplit


def _ap(t, extra_offset, dims):
    """AP over tile t's tensor: partition dim copied from t, free dims given
    explicitly as [step, count] (steps in elements)."""
    return bass.AP(tensor=t.tensor, offset=t.offset + extra_offset, ap=[t.ap[0]] + dims)


def _build_nc(split_waits=True):
    nc = bass.Bass()
    z_in = nc.dram_tensor("z", [P, T * D], F32, kind="ExternalInput")
    basis_in = nc.dram_tensor("basis", [D], F32, kind="ExternalInput")
    zq_out = nc.dram_tensor("zq", [P, T * D], F32, kind="ExternalOutput")
    idx_out = nc.dram_tensor("idx", [P, T], I32, kind="ExternalOutput")
    sums_out = nc.dram_tensor("sums", [P, 3], F32, kind="ExternalOutput")
    avgp_out = nc.dram_tensor("avgp", [64, 32], F32, kind="ExternalOutput")

    with tile.TileContext(nc) as tc:
        with (
            tc.tile_pool(name="main", bufs=1) as pool,
            tc.tile_pool(name="psum", bufs=1, space="PSUM") as psum,
        ):
            zt = pool.tile([P, T, D], F32)          # [p, t, d]
            nc.sync.dma_start(out=zt, in_=z_in[:, :].rearrange("p (t d) -> p t d", d=D))
            basis2 = pool.tile([P, D], F32)          # 2^(15-d), replicated rows
            nc.sync.dma_start(
                out=basis2,
                in_=bass.AP(tensor=basis_in, offset=0, ap=[[0, P], [1, D]]),
            )

            s1 = pool.tile([P, T, D], F32)          # scratch
            sums = pool.tile([P, 3], F32)

            # ---- normalization chain, split into two t-halves so the
            # first half's sigmoid (and therefore the tree + PE stream)
            # starts while the second half is still normalizing ----
            H = T // 2
            ssq = pool.tile([P, T], F32)
            ln_s = pool.tile([P, T], F32)
            rn = pool.tile([P, T], F32)
            zn = pool.tile([P, T, D], F32)
            pq = pool.tile([P, 2, D, T], F32)       # [p, sign(0:-,1:+), d, t]
            halves = [(h * H, H) for h in range(2)]
            for lo, n in halves:
                nc.scalar.activation(s1[:, lo:lo + n], zt[:, lo:lo + n], AF.Square)
            for lo, n in halves:
                nc.vector.tensor_reduce(
                    out=ssq[:, lo:lo + n], in_=s1[:, lo:lo + n],
                    axis=mybir.AxisListType.X, op=ALU.add,
                )
            rn_acts = []
            for lo, n in halves:
                nc.scalar.activation(ln_s[:, lo:lo + n], ssq[:, lo:lo + n], AF.Ln)
                rn_acts.append(nc.scalar.activation(
                    rn[:, lo:lo + n], ln_s[:, lo:lo + n], AF.Exp, scale=-0.5
                ))
            zn_halves = []
            for lo, n in halves:
                zn_halves.append(nc.vector.tensor_mul(
                    zn[:, lo:lo + n], zt[:, lo:lo + n],
                    _ap(rn, lo, [[1, n], [0, D]]),
                ))
            zn_i = zn_halves[-1]
            # ---- sigmoid set ----
            sigs = []
            for h, (lo, n) in enumerate(halves):
                sig = nc.scalar.activation(
                    _ap(pq, D * T + lo, [[1, n], [T, D]]),
                    zn[:, lo:lo + n], AF.Sigmoid, scale=4.0 * Q,
                )
                sigs.append(sig)
                # p_minus = 1 - p_plus on DVE (2x)
                nc.vector.tensor_scalar(
                    out=_ap(pq, lo, [[T, D], [1, n]]),
                    in0=_ap(pq, D * T + lo, [[T, D], [1, n]]),
                    scalar1=-1.0, scalar2=1.0, op0=ALU.mult, op1=ALU.add,
                )
            # keep the natural_log block (h2's Ln/Exp) ahead of the sigmoids
            bass._add_dep_helper(
                sigs[0].ins, rn_acts[-1].ins, sync=False, reason="act set order"
            )
            sig_p = sigs[-1]

            # sign path on DVE: fills the window while ACT runs the sigmoid
            cmp = pool.tile([P, T, D], FP16)        # 1.0 if z>0 else 0.0
            nc.vector.tensor_scalar(
                out=cmp, in0=zt, scalar1=0.0, scalar2=None, op0=ALU.is_gt
            )
            zqt = pool.tile([P, T, D], F32)
            nc.vector.tensor_scalar(
                out=zqt, in0=cmp, scalar1=2.0 * Q, scalar2=-Q,
                op0=ALU.mult, op1=ALU.add,
            )
            nc.sync.dma_start(
                out=zq_out[:, :].rearrange("p (t d) -> p t d", d=D), in_=zqt
            )

            # ---- Bernoulli product tree + avg_prob matmuls, chunked ----
            # pq element (s, d, t) at s*D*T + d*T + t; tree tiles are t-inner
            t16 = pool.tile([P, 32, T], FP16)       # [p, (g,i4), t]
            t32 = pool.tile([P, 64, T], FP16)       # [p, (g,j5), t]
            p01 = pool.tile([P, 2, 4, T], FP16)     # [p, g, (b0,b1), t]
            p23 = pool.tile([P, 2, 4, T], FP16)
            q45 = pool.tile([P, 2, 4, T], FP16)
            q67 = pool.tile([P, 2, 4, T], FP16)
            q678 = pool.tile([P, 2, 8, T], FP16)
            pss = [psum.tile([64, 32], F32, name=f"ps{c}") for c in range(len(CHUNKS))]

            def pair(dst, d_hi, d_lo, g, lo, n):
                """dst[g, b_hi, b_lo, t] = pq[b_hi, d_hi, t] * pq[b_lo, d_lo, t]"""
                nc.vector.tensor_mul(
                    _ap(dst, g * 4 * T + lo, [[2 * T, 2], [T, 2], [1, n]]),
                    _ap(pq, (9 * g + d_hi) * T + lo, [[D * T, 2], [0, 2], [1, n]]),
                    _ap(pq, (9 * g + d_lo) * T + lo, [[0, 2], [D * T, 2], [1, n]]),
                )

            # tree stages: a small t-prefix first so the PE matmul stream
            # starts early, then the bulk in one low-overhead pass
            C0 = CHUNKS[0][1]

            def smalls(lo, n):
                for g in range(2):
                    pair(p01, 0, 1, g, lo, n)
                    pair(p23, 2, 3, g, lo, n)
                    pair(q45, 4, 5, g, lo, n)
                    pair(q67, 6, 7, g, lo, n)
                    # q678[(i67, b8), t] = q67[i67, t] * pq[b8, d=9g+8, t]
                    nc.vector.tensor_mul(
                        _ap(q678, g * 8 * T + lo, [[2 * T, 4], [T, 2], [1, n]]),
                        _ap(q67, g * 4 * T + lo, [[T, 4], [0, 2], [1, n]]),
                        _ap(pq, (9 * g + 8) * T + lo, [[0, 4], [D * T, 2], [1, n]]),
                    )

            def finals(c0, cs):
                for g in range(2):
                    # t16[(g, i01, i23), t] = p01[i01, t] * p23[i23, t]
                    nc.vector.tensor_mul(
                        _ap(t16, g * 16 * T + c0, [[4 * T, 4], [T, 4], [1, cs]]),
                        _ap(p01, g * 4 * T + c0, [[T, 4], [0, 4], [1, cs]]),
                        _ap(p23, g * 4 * T + c0, [[0, 4], [T, 4], [1, cs]]),
                    )
                    # t32[(g, i45, i678), t] = q45[i45, t] * q678[i678, t]
                    last = nc.vector.tensor_mul(
                        _ap(t32, g * 32 * T + c0, [[8 * T, 4], [T, 8], [1, cs]]),
                        _ap(q45, g * 4 * T + c0, [[T, 4], [0, 8], [1, cs]]),
                        _ap(q678, g * 8 * T + c0, [[0, 4], [T, 8], [1, cs]]),
                    )
                return last

            def mms(c, c0, cs):
                for tt in range(c0, c0 + cs):
                    nc.tensor.matmul(
                        pss[c],
                        _ap(t32, tt, [[T, 64]]),   # [K=128, 64] stationary
                        _ap(t16, tt, [[T, 32]]),   # [K=128, 32] moving
                        start=(tt == c0),
                        stop=(tt == c0 + cs - 1),
                    )

            smalls(0, C0)
            finals(0, C0)
            mms(0, 0, C0)
            smalls(CHUNKS[1][0], CHUNKS[1][1])
            finals(CHUNKS[1][0], CHUNKS[1][1])
            mms(1, CHUNKS[1][0], CHUNKS[1][1])
            smalls(CHUNKS[2][0], T - CHUNKS[2][0])
            last_fin = None
            for c, (c0, cs) in enumerate(CHUNKS):
                if c < 2:
                    continue
                last_fin = finals(c0, cs)
                mms(c, c0, cs)

            # ---- entropy + commit partial sums ----
            if USE_SILU:
                sl1 = pool.tile([P, D, T], F32)
                silu_i = nc.scalar.activation(      # sum of x*sigmoid(x), x=4Q*zn
                    sl1, zn, AF.Silu, scale=4.0 * Q, accum_out=sums[:, 1:2]
                )
                bass._add_dep_helper(
                    silu_i.ins, sig_p.ins, sync=False, reason="act table set order"
                )
            ex = pool.tile([P, D, T], F32)
            exp_e = nc.scalar.activation(ex, zn, AF.Exp, scale=4.0 * Q)   # e^x
            nc.scalar.activation(                                          # softplus
                s1, ex, AF.Ln, bias=1.0, accum_out=sums[:, 0:1]
            )
            ab2 = pool.tile([P, D, T], F32)
            nc.scalar.activation(                                          # 2Q|zn|
                ab2, zn, AF.Abs, scale=2.0 * Q, accum_out=sums[:, 2:3]
            )
            # keep Exp/Ln after the sigmoid/silu block (one table switch only)
            bass._add_dep_helper(
                exp_e.ins,
                (silu_i if USE_SILU else sig_p).ins,
                sync=False,
                reason="act table set order",
            )
            if not USE_SILU:
                # sum of zn*sigmoid(x) via DVE (sim has no Silu)
                sw1 = pool.tile([P, T, D], F32)
                nc.vector.tensor_mul(
                    sw1, zn, _ap(pq, D * T, [[1, T], [T, D]])
                )
                nc.vector.tensor_reduce(
                    out=sums[:, 1:2], in_=sw1, axis=mybir.AxisListType.XY, op=ALU.add
                )

            # idx = sum_d 2^(17-d)*cmp_d  (exact: powers of two)
            sw2 = pool.tile([P, T, D], FP16)
            idx_i = nc.vector.tensor_mul(sw2, cmp, _ap(basis2, 0, [[0, T], [1, D]]))
            bass._add_dep_helper(
                idx_i.ins, last_fin.ins, sync=False, reason="idx in DVE tail window"
            )
            idxf = pool.tile([P, T], F32)
            nc.vector.tensor_reduce(
                out=idxf, in_=sw2, axis=mybir.AxisListType.X, op=ALU.add
            )
            idxi = pool.tile([P, T], I32)
            nc.vector.tensor_scalar(
                out=idxi, in0=idxf, scalar1=4.0, scalar2=None, op0=ALU.mult
            )
            nc.sync.dma_start(out=idx_out[:, :], in_=idxi)

            avg_sb = pool.tile([64, 32], F32)
            nc.scalar.copy(avg_sb, pss[0])
            for c in range(1, len(CHUNKS)):
                nc.vector.tensor_add(avg_sb, avg_sb, pss[c])
            nc.sync.dma_start(out=avgp_out[:, :], in_=avg_sb)
            nc.sync.dma_start(out=sums_out[:, :], in_=sums)

    if split_waits:
        _split_waits(nc)
    return nc


_NC_CACHE = None


def _get_nc():
    global _NC_CACHE
    if _NC_CACHE is None:
        _NC_CACHE = _build_nc()
    return _NC_CACHE


_BASIS2 = (2.0 ** np.arange(D - 3, -3, -1)).astype(np.float32)  # 2^(15-d), fp16-safe


def _shard(z):
    """z [B, L, D] f32 -> list of per-core input dicts (device layout)."""
    zf = np.ascontiguousarray(z, dtype=np.float32).reshape(N, D)
    ins = []
    for c in range(N_CORES):
        zc = zf[c * NC_SAMP : (c + 1) * NC_SAMP]          # [16384, 18]
        ins.append({"z": zc.reshape(P, T * D), "basis": _BASIS2})
    return ins


def _unshard(results):
    zq = np.empty((N, D), np.float32)
    idx = np.empty((N,), np.int32)
    sp_t = sw_t = cm_t = 0.0
    avg = np.zeros((64, 32), np.float64)
    for c, r in enumerate(results):
        zq[c * NC_SAMP : (c + 1) * NC_SAMP] = r["zq"].reshape(NC_SAMP, D)
        idx[c * NC_SAMP : (c + 1) * NC_SAMP] = r["idx"].reshape(NC_SAMP)
        s = r["sums"].astype(np.float64)
        sp_t += s[:, 0].sum()
        sw_t += s[:, 1].sum()
        cm_t += s[:, 2].sum()
        avg += r["avgp"].astype(np.float64)

    persample = (sp_t - (1.0 if USE_SILU else 4.0 * Q) * sw_t) / N
    commit = 2.0 - cm_t / N
    ap = np.empty((2, 512), np.float64)
    for g in range(2):
        # avg[(g,j), (g,i)] = sum_n b_j * a_i ; ap[g, i*32+j] = that / N
        ap[g] = (avg[g * 32 : (g + 1) * 32, g * 16 : (g + 1) * 16].T / N).reshape(512)
    cb_ent = float(-(ap * np.log(ap + 1e-8)).sum())
    total = commit + persample - cb_ent

    return (
        zq.reshape(B, L, D),
        np.float32(total),
        np.float32(persample),
        np.float32(cb_ent),
        np.float32(commit),
        idx.reshape(B, L),
    )


def kernel(z):
    nc = _get_nc()
    ins = _shard(np.asarray(z))
    res = run_bass_kernel_spmd(nc, ins, core_ids=list(range(N_CORES)))
    return _unshard(res.results)


# revision 46
# speedup vs baseline: 1.1331x; 1.0018x over previous
"""BSQ quantizer kernel for Trainium2, data-parallel across 8 NeuronCores.

Math notes (vs the JAX reference):
  - zq = sign(z) * Q (the straight-through estimator is a numerical no-op in
    the forward pass; sign(z/||z||) == sign(z)).
  - The per-group softmax over 512 sign codes factorizes into per-bit
    Bernoullis: prob_d = prod_c sigmoid(4*Q*s_dc*zn_c).  So
    avg_prob[g] = mean_n softmaxA_n (x) softmaxB_n with A = first 4 bits
    (16 values), B = last 5 bits (32 values); each side is a product of
    per-bit probabilities (automatically normalized), and the sum over
    samples of the outer product is a matmul contracted over samples.
  - per-bit entropy: -(p log p + (1-p) log(1-p)) with p = sigmoid(+-x)
    equals softplus(x) - x*sigmoid(x), x = 4*Q*zn.
  - commit loss per sample: sum_d (Q*sign - zn_d)^2 = 2 - 2*Q*sum_d |zn_d|.

Per-core layout: 16384 samples as [p=128 partitions, t=128 chunks, d=18],
sample n = p*128 + t, so sharding is a pure reshape on the host.
"""

import math

import numpy as np

import bass_rust
import concourse.bass as bass
import concourse.tile as tile
from concourse import mybir
from concourse.bass_utils import run_bass_kernel_spmd

AF = mybir.ActivationFunctionType
ALU = mybir.AluOpType
F32 = mybir.dt.float32
BF16 = mybir.dt.bfloat16
FP16 = mybir.dt.float16
I32 = mybir.dt.int32

N_CORES = 8
B, L, D = 32, 4096, 18
N = B * L                    # 131072 samples
NC_SAMP = N // N_CORES       # 16384 per core
P = 128                      # partitions
T = NC_SAMP // P             # 128 t-chunks per core
Q = 1.0 / math.sqrt(D)
# (start, size) chunks of t for the final tree stages + matmuls; the last
# chunk is small to shorten the PE tail after the last DVE op
CHUNKS = [(0, 16), (16, 52), (68, 52), (120, 8)]
IDX_OFF = (2.0**D - 1.0) / 2.0   # 131071.5
USE_SILU = True                  # HW has Silu; CoreSim doesn't


def _split_waits(nc, maxw=1):
    """walrus in this container only lowers one sync-wait per instruction;
    move excess waits onto freshly inserted NOPs just before the offender."""
    nsplit = 0
    for bb in nc.main_func.blocks:
        lst = bb.instructions
        i = 0
        while i < len(lst):
            ins = lst[i]
            si = ins.sync_info
            if si is not None and len(si.on_wait) > maxw:
                waits = list(si.on_wait)
                extra = waits[:-maxw]
                si.on_wait = waits[-maxw:]
                nops = []
                for j in range(0, len(extra), maxw):
                    nsplit += 1
                    nop = mybir.InstNoOp(name=f"I-waitsplit-{nsplit}")
                    nop.engine = ins.engine
                    nop.sync_info = bass_rust.SyncInfo(
                        on_wait=extra[j : j + maxw], on_update=[]
                    )
                    nops.append(nop)
                for k, nop in enumerate(nops):
                    lst.insert(i + k, nop)
                i += len(nops)
            i += 1
    return nsplit


def _ap(t, extra_offset, dims):
    """AP over tile t's tensor: partition dim copied from t, free dims given
    explicitly as [step, count] (steps in elements)."""
    return bass.AP(tensor=t.tensor, offset=t.offset + extra_offset, ap=[t.ap[0]] + dims)


def _build_nc(split_waits=True):
    nc = bass.Bass()
    z_in = nc.dram_tensor("z", [P, T * D], F32, kind="ExternalInput")
    basis_in = nc.dram_tensor("basis", [D], F32, kind="ExternalInput")
    zq_out = nc.dram_tensor("zq", [P, T * D], F32, kind="ExternalOutput")
    idx_out = nc.dram_tensor("idx", [P, T], I32, kind="ExternalOutput")
    sums_out = nc.dram_tensor("sums", [P, 3], F32, kind="ExternalOutput")
    avgp_out = nc.dram_tensor("avgp", [64, 32], F32, kind="ExternalOutput")

    with tile.TileContext(nc) as tc:
        with (
            tc.tile_pool(name="main", bufs=1) as pool,
            tc.tile_pool(name="psum", bufs=1, space="PSUM") as psum,
        ):
            zt = pool.tile([P, T, D], F32)          # [p, t, d]
            nc.sync.dma_start(out=zt, in_=z_in[:, :].rearrange("p (t d) -> p t d", d=D))
            basis2 = pool.tile([P, D], F32)          # 2^(15-d), replicated rows
            nc.sync.dma_start(
                out=basis2,
                in_=bass.AP(tensor=basis_in, offset=0, ap=[[0, P], [1, D]]),
            )

            s1 = pool.tile([P, T, D], F32)          # scratch
            sums = pool.tile([P, 3], F32)

            # ---- normalization chain, split into two t-halves so the
            # first half's sigmoid (and therefore the tree + PE stream)
            # starts while the second half is still normalizing ----
            H = T // 2
            ssq = pool.tile([P, T], F32)
            ln_s = pool.tile([P, T], F32)
            rn = pool.tile([P, T], F32)
            zn = pool.tile([P, T, D], F32)
            pq = pool.tile([P, 2, D, T], F32)       # [p, sign(0:-,1:+), d, t]
            halves = [(h * H, H) for h in range(2)]
            for lo, n in halves:
                nc.scalar.activation(s1[:, lo:lo + n], zt[:, lo:lo + n], AF.Square)
            for lo, n in halves:
                nc.vector.tensor_reduce(
                    out=ssq[:, lo:lo + n], in_=s1[:, lo:lo + n],
                    axis=mybir.AxisListType.X, op=ALU.add,
                )
            rn_acts = []
            for lo, n in halves:
                nc.scalar.activation(ln_s[:, lo:lo + n], ssq[:, lo:lo + n], AF.Ln)
                rn_acts.append(nc.scalar.activation(
                    rn[:, lo:lo + n], ln_s[:, lo:lo + n], AF.Exp, scale=-0.5
                ))
            zn_halves = []
            for lo, n in halves:
                zn_halves.append(nc.vector.tensor_mul(
                    zn[:, lo:lo + n], zt[:, lo:lo + n],
                    _ap(rn, lo, [[1, n], [0, D]]),
                ))
            zn_i = zn_halves[-1]
            # ---- sigmoid set ----
            sigs = []
            for h, (lo, n) in enumerate(halves):
                sig = nc.scalar.activation(
                    _ap(pq, D * T + lo, [[1, n], [T, D]]),
                    zn[:, lo:lo + n], AF.Sigmoid, scale=4.0 * Q,
                )
                sigs.append(sig)
                # p_minus = 1 - p_plus on DVE (2x)
                nc.vector.tensor_scalar(
                    out=_ap(pq, lo, [[T, D], [1, n]]),
                    in0=_ap(pq, D * T + lo, [[T, D], [1, n]]),
                    scalar1=-1.0, scalar2=1.0, op0=ALU.mult, op1=ALU.add,
                )
            # keep the natural_log block (h2's Ln/Exp) ahead of the sigmoids
            bass._add_dep_helper(
                sigs[0].ins, rn_acts[-1].ins, sync=False, reason="act set order"
            )
            sig_p = sigs[-1]

            # sign path on DVE: fills the window while ACT runs the sigmoid
            cmp = pool.tile([P, T, D], FP16)        # 1.0 if z>0 else 0.0
            nc.vector.tensor_scalar(
                out=cmp, in0=zt, scalar1=0.0, scalar2=None, op0=ALU.is_gt
            )
            zqt = pool.tile([P, T, D], F32)
            zq_i = nc.vector.tensor_scalar(
                out=zqt, in0=cmp, scalar1=2.0 * Q, scalar2=-Q,
                op0=ALU.mult, op1=ALU.add,
            )
            nc.sync.dma_start(
                out=zq_out[:, :].rearrange("p (t d) -> p t d", d=D), in_=zqt
            )

            # ---- Bernoulli product tree + avg_prob matmuls, chunked ----
            # pq element (s, d, t) at s*D*T + d*T + t; tree tiles are t-inner
            t16 = pool.tile([P, 32, T], FP16)       # [p, (g,i4), t]
            t32 = pool.tile([P, 64, T], FP16)       # [p, (g,j5), t]
            p01 = pool.tile([P, 2, 4, T], FP16)     # [p, g, (b0,b1), t]
            p23 = pool.tile([P, 2, 4, T], FP16)
            q45 = pool.tile([P, 2, 4, T], FP16)
            q67 = pool.tile([P, 2, 4, T], FP16)
            q678 = pool.tile([P, 2, 8, T], FP16)
            pss = [psum.tile([64, 32], F32, name=f"ps{c}") for c in range(len(CHUNKS))]

            def pair(dst, d_hi, d_lo, g, lo, n):
                """dst[g, b_hi, b_lo, t] = pq[b_hi, d_hi, t] * pq[b_lo, d_lo, t]"""
                nc.vector.tensor_mul(
                    _ap(dst, g * 4 * T + lo, [[2 * T, 2], [T, 2], [1, n]]),
                    _ap(pq, (9 * g + d_hi) * T + lo, [[D * T, 2], [0, 2], [1, n]]),
                    _ap(pq, (9 * g + d_lo) * T + lo, [[0, 2], [D * T, 2], [1, n]]),
                )

            # tree stages: a small t-prefix first so the PE matmul stream
            # starts early, then the bulk in one low-overhead pass
            C0 = CHUNKS[0][1]

            def smalls(lo, n):
                for g in range(2):
                    pair(p01, 0, 1, g, lo, n)
                    pair(p23, 2, 3, g, lo, n)
                    pair(q45, 4, 5, g, lo, n)
                    pair(q67, 6, 7, g, lo, n)
                    # q678[(i67, b8), t] = q67[i67, t] * pq[b8, d=9g+8, t]
                    nc.vector.tensor_mul(
                        _ap(q678, g * 8 * T + lo, [[2 * T, 4], [T, 2], [1, n]]),
                        _ap(q67, g * 4 * T + lo, [[T, 4], [0, 2], [1, n]]),
                        _ap(pq, (9 * g + 8) * T + lo, [[0, 4], [D * T, 2], [1, n]]),
                    )

            def finals(c0, cs):
                for g in range(2):
                    # t16[(g, i01, i23), t] = p01[i01, t] * p23[i23, t]
                    nc.vector.tensor_mul(
                        _ap(t16, g * 16 * T + c0, [[4 * T, 4], [T, 4], [1, cs]]),
                        _ap(p01, g * 4 * T + c0, [[T, 4], [0, 4], [1, cs]]),
                        _ap(p23, g * 4 * T + c0, [[0, 4], [T, 4], [1, cs]]),
                    )
                    # t32[(g, i45, i678), t] = q45[i45, t] * q678[i678, t]
                    last = nc.vector.tensor_mul(
                        _ap(t32, g * 32 * T + c0, [[8 * T, 4], [T, 8], [1, cs]]),
                        _ap(q45, g * 4 * T + c0, [[T, 4], [0, 8], [1, cs]]),
                        _ap(q678, g * 8 * T + c0, [[0, 4], [T, 8], [1, cs]]),
                    )
                return last

            def mms(c, c0, cs):
                for tt in range(c0, c0 + cs):
                    nc.tensor.matmul(
                        pss[c],
                        _ap(t32, tt, [[T, 64]]),   # [K=128, 64] stationary
                        _ap(t16, tt, [[T, 32]]),   # [K=128, 32] moving
                        start=(tt == c0),
                        stop=(tt == c0 + cs - 1),
                    )

            smalls(0, C0)
            finals(0, C0)
            mms(0, 0, C0)
            smalls(CHUNKS[1][0], CHUNKS[1][1])
            fin1 = finals(CHUNKS[1][0], CHUNKS[1][1])
            mms(1, CHUNKS[1][0], CHUNKS[1][1])
            bass._add_dep_helper(
                zq_i.ins, fin1.ins, sync=False, reason="zq after chunk1 tree"
            )
            smalls(CHUNKS[2][0], T - CHUNKS[2][0])
            last_fin = None
            for c, (c0, cs) in enumerate(CHUNKS):
                if c < 2:
                    continue
                last_fin = finals(c0, cs)
                mms(c, c0, cs)

            # ---- entropy + commit partial sums ----
            if USE_SILU:
                sl1 = pool.tile([P, D, T], F32)
                silu_i = nc.scalar.activation(      # sum of x*sigmoid(x), x=4Q*zn
                    sl1, zn, AF.Silu, scale=4.0 * Q, accum_out=sums[:, 1:2]
                )
                bass._add_dep_helper(
                    silu_i.ins, sig_p.ins, sync=False, reason="act table set order"
                )
            ex = pool.tile([P, D, T], F32)
            exp_e = nc.scalar.activation(ex, zn, AF.Exp, scale=4.0 * Q)   # e^x
            nc.scalar.activation(                                          # softplus
                s1, ex, AF.Ln, bias=1.0, accum_out=sums[:, 0:1]
            )
            ab2 = pool.tile([P, D, T], F32)
            nc.scalar.activation(                                          # 2Q|zn|
                ab2, zn, AF.Abs, scale=2.0 * Q, accum_out=sums[:, 2:3]
            )
            # keep Exp/Ln after the sigmoid/silu block (one table switch only)
            bass._add_dep_helper(
                exp_e.ins,
                (silu_i if USE_SILU else sig_p).ins,
                sync=False,
                reason="act table set order",
            )
            if not USE_SILU:
                # sum of zn*sigmoid(x) via DVE (sim has no Silu)
                sw1 = pool.tile([P, T, D], F32)
                nc.vector.tensor_mul(
                    sw1, zn, _ap(pq, D * T, [[1, T], [T, D]])
                )
                nc.vector.tensor_reduce(
                    out=sums[:, 1:2], in_=sw1, axis=mybir.AxisListType.XY, op=ALU.add
                )

            # idx = sum_d 2^(17-d)*cmp_d  (exact: powers of two)
            sw2 = pool.tile([P, T, D], FP16)
            idx_i = nc.vector.tensor_mul(sw2, cmp, _ap(basis2, 0, [[0, T], [1, D]]))
            bass._add_dep_helper(
                idx_i.ins, last_fin.ins, sync=False, reason="idx in DVE tail window"
            )
            idxf = pool.tile([P, T], F32)
            nc.vector.tensor_reduce(
                out=idxf, in_=sw2, axis=mybir.AxisListType.X, op=ALU.add
            )
            idxi = pool.tile([P, T], I32)
            nc.vector.tensor_scalar(
                out=idxi, in0=idxf, scalar1=4.0, scalar2=None, op0=ALU.mult
            )
            nc.sync.dma_start(out=idx_out[:, :], in_=idxi)

            avg_sb = pool.tile([64, 32], F32)
            nc.scalar.copy(avg_sb, pss[0])
            for c in range(1, len(CHUNKS)):
                nc.vector.tensor_add(avg_sb, avg_sb, pss[c])
            nc.sync.dma_start(out=avgp_out[:, :], in_=avg_sb)
            nc.sync.dma_start(out=sums_out[:, :], in_=sums)

    if split_waits:
        _split_waits(nc)
    return nc


_NC_CACHE = None


def _get_nc():
    global _NC_CACHE
    if _NC_CACHE is None:
        _NC_CACHE = _build_nc()
    return _NC_CACHE


_BASIS2 = (2.0 ** np.arange(D - 3, -3, -1)).astype(np.float32)  # 2^(15-d), fp16-safe


def _shard(z):
    """z [B, L, D] f32 -> list of per-core input dicts (device layout)."""
    zf = np.ascontiguousarray(z, dtype=np.float32).reshape(N, D)
    ins = []
    for c in range(N_CORES):
        zc = zf[c * NC_SAMP : (c + 1) * NC_SAMP]          # [16384, 18]
        ins.append({"z": zc.reshape(P, T * D), "basis": _BASIS2})
    return ins


def _unshard(results):
    zq = np.empty((N, D), np.float32)
    idx = np.empty((N,), np.int32)
    sp_t = sw_t = cm_t = 0.0
    avg = np.zeros((64, 32), np.float64)
    for c, r in enumerate(results):
        zq[c * NC_SAMP : (c + 1) * NC_SAMP] = r["zq"].reshape(NC_SAMP, D)
        idx[c * NC_SAMP : (c + 1) * NC_SAMP] = r["idx"].reshape(NC_SAMP)
        s = r["sums"].astype(np.float64)
        sp_t += s[:, 0].sum()
        sw_t += s[:, 1].sum()
        cm_t += s[:, 2].sum()
        avg += r["avgp"].astype(np.float64)

    persample = (sp_t - (1.0 if USE_SILU else 4.0 * Q) * sw_t) / N
    commit = 2.0 - cm_t / N
    ap = np.empty((2, 512), np.float64)
    for g in range(2):
        # avg[(g,j), (g,i)] = sum_n b_j * a_i ; ap[g, i*32+j] = that / N
        ap[g] = (avg[g * 32 : (g + 1) * 32, g * 16 : (g + 1) * 16].T / N).reshape(512)
    cb_ent = float(-(ap * np.log(ap + 1e-8)).sum())
    total = commit + persample - cb_ent

    return (
        zq.reshape(B, L, D),
        np.float32(total),
        np.float32(persample),
        np.float32(cb_ent),
        np.float32(commit),
        idx.reshape(B, L),
    )


def kernel(z):
    nc = _get_nc()
    ins = _shard(np.asarray(z))
    res = run_bass_kernel_spmd(nc, ins, core_ids=list(range(N_CORES)))
    return _unshard(res.results)
